# revision 6
# baseline (speedup 1.0000x reference)
"""Trainium2 Bass kernel for nn_BlockSparseMoE (top-2 of 8 experts, SwiGLU).

Strategy (expert-parallel, sparse dispatch):
  - Host: compute router (gate matmul + softmax + top-2 + renorm) in fp64,
    gather each expert's tokens into a capacity-padded batch (the
    "all-to-all dispatch by selected expert" happens at input-sharding
    time, which is host-side by construction).
  - Device (SPMD x8, one expert per core): xT [D, C] bf16 streams through
    w1/w3 (SwiGLU) and w2 in bf16 with fp32 PSUM accumulation, rows scaled
    by the renormalized top-2 weight. No collectives needed.
  - Host: scatter-add the two expert contributions per token.

Layout per core:
  phase A: hT[f, t] = silu(x@w1)^T * (x@w3)^T computed directly transposed
           (lhsT = w1 d-chunk [128, 128f], rhs = xT d-chunk [128, tchunk])
           so no on-device transposes are ever needed.
  phase B: y[t, d] accumulated over 32 f-chunks (lhsT = hT f-chunk, rhs =
           w2 f-chunk), scaled by combine weight via per-partition scalar.
"""

import numpy as np
import ml_dtypes

HIDDEN = 1024
FFN = 4096
NUM_EXPERTS = 8
TOP_K = 2
N_CORES = 8

_BF16 = ml_dtypes.bfloat16
_nc_cache = {}


# ---------------------------------------------------------------- router ----
def _route(x, gate_w, gate_b):
    """Top-2 routing. Returns per-expert (token_idx, renorm_weight)."""
    logits = x.astype(np.float64) @ gate_w.astype(np.float64) + gate_b.astype(
        np.float64
    )
    logits -= logits.max(axis=-1, keepdims=True)
    p = np.exp(logits)
    p /= p.sum(axis=-1, keepdims=True)
    # top-2 by prob, ties broken by lower index (matches jax.lax.top_k)
    top2 = np.argsort(-p, axis=-1, kind="stable")[:, :TOP_K]
    pt = np.take_along_axis(p, top2, axis=-1)
    wt = pt / pt.sum(axis=-1, keepdims=True)
    idxs, wts = [], []
    for e in range(NUM_EXPERTS):
        mask = top2 == e  # [T, 2]
        tok = np.nonzero(mask.any(axis=-1))[0]
        w = wt[tok, np.argmax(mask[tok], axis=-1)]
        idxs.append(tok)
        wts.append(w.astype(np.float32))
    return idxs, wts


# ------------------------------------------------------------- device IR ----
def _build(C, chunk):
    """Build the per-core Bacc graph for capacity C (= chunk * n_chunks)."""
    import concourse.bacc as bacc
    import concourse.bass as bass
    import concourse.mybir as mybir
    import concourse.tile as tile

    n_chunks = C // chunk
    n_subs = chunk // 128  # 128-token sub-tiles per chunk
    DC = HIDDEN // 128  # 8 contraction chunks for x@w1
    FT = FFN // 128  # 32 f-tiles
    FG = FFN // 512  # 8 f-groups (512 wide)
    DO = HIDDEN // 512  # 2 output-d chunks

    bf16 = mybir.dt.bfloat16
    f32 = mybir.dt.float32

    nc = bacc.Bacc("TRN2", target_bir_lowering=False, debug=False,
                   num_devices=N_CORES)

    xT_d = nc.dram_tensor("xT", [HIDDEN, C], bf16, kind="ExternalInput")
    w1_d = nc.dram_tensor("w1", [HIDDEN, FFN], bf16, kind="ExternalInput")
    w3_d = nc.dram_tensor("w3", [HIDDEN, FFN], bf16, kind="ExternalInput")
    w2_d = nc.dram_tensor("w2", [FFN, HIDDEN], bf16, kind="ExternalInput")
    s_d = nc.dram_tensor("s", [C], f32, kind="ExternalInput")
    y_d = nc.dram_tensor("y", [C, HIDDEN], f32, kind="ExternalOutput")

    # DRAM views tiled for 128-partition DMA
    xT_v = xT_d.ap().rearrange("(dc p) c -> p dc c", p=128)
    w1_v = w1_d.ap().rearrange("(dc p) f -> p dc f", p=128)
    w3_v = w3_d.ap().rearrange("(dc p) f -> p dc f", p=128)
    w2_v = w2_d.ap().rearrange("(ft p) d -> p ft d", p=128)
    s_v = s_d.ap().rearrange("(j p) -> p j", p=128)

    with tile.TileContext(nc) as tc:
        with (
            tc.tile_pool(name="res", bufs=1) as res,
            tc.tile_pool(name="w13", bufs=2) as w13,
            tc.tile_pool(name="hp", bufs=2) as hp,
            tc.tile_pool(name="sil", bufs=4) as silp,
            tc.tile_pool(name="yo", bufs=4) as yop,
            tc.tile_pool(name="ps", bufs=2, space=bass.MemorySpace.PSUM) as ps,
            tc.tile_pool(name="yps", bufs=4, space=bass.MemorySpace.PSUM) as yps,
        ):
            # resident tensors.  DMA order matters: the PE's first work is
            # chunk-0 phase A, which needs xT[:, :, :chunk] and w1/w3 of
            # fg 0 — so those transfers go first; the 8MB w2 (needed only
            # for phase B, ~85us in) is interleaved across chunk-0's
            # f-group loop so it never blocks the critical path.
            xT = res.tile([128, DC, C], bf16, tag="xT")
            w2 = res.tile([128, FT, HIDDEN], bf16, tag="w2")
            s_sb = res.tile([128, C // 128], f32, tag="s")
            # chunk-0 xT in dc-halves so the first matmuls (dc 0..3) can
            # start before the whole slice lands
            nc.sync.dma_start(xT[:, 0:4, 0:chunk], xT_v[:, 0:4, 0:chunk])
            nc.sync.dma_start(xT[:, 4:DC, 0:chunk], xT_v[:, 4:DC, 0:chunk])

            for t in range(n_chunks):
                t0 = t * chunk
                hT = hp.tile([128, FT, chunk], bf16, tag="hT")
                # ---- phase A: hT[f, t] for this token chunk ----
                for fg in range(FG):
                    w1_sb = w13.tile([128, DC, 512], bf16, tag="w1")
                    w3_sb = w13.tile([128, DC, 512], bf16, tag="w3")
                    fsl = slice(fg * 512, (fg + 1) * 512)
                    if t == 0 and fg == 0:
                        # split the very first weight loads so the PE's
                        # first accumulation (dc 0..3) starts ASAP
                        nc.sync.dma_start(w1_sb[:, 0:4, :], w1_v[:, 0:4, fsl])
                        nc.sync.dma_start(w1_sb[:, 4:DC, :], w1_v[:, 4:DC, fsl])
                        nc.sync.dma_start(w3_sb[:, 0:4, :], w3_v[:, 0:4, fsl])
                        nc.sync.dma_start(w3_sb[:, 4:DC, :], w3_v[:, 4:DC, fsl])
                    else:
                        nc.sync.dma_start(w1_sb[:], w1_v[:, :, fsl])
                        nc.sync.dma_start(w3_sb[:], w3_v[:, :, fsl])
                    if t == 0:
                        # stream w2 (4 ft-rows = 1MB per fg) behind the
                        # critical w1/w3 loads; complete before phase B
                        nc.sync.dma_start(w2[:, fg * 4:(fg + 1) * 4, :],
                                          w2_v[:, fg * 4:(fg + 1) * 4, :])
                        if fg == 0:
                            nc.sync.dma_start(s_sb[:], s_v)
                        if fg + 1 < n_chunks:
                            # prefetch next chunk's xT slice
                            tn = fg + 1
                            nc.sync.dma_start(
                                xT[:, :, tn * chunk:(tn + 1) * chunk],
                                xT_v[:, :, tn * chunk:(tn + 1) * chunk],
                            )
                    for fl in range(4):
                        ft = fg * 4 + fl
                        ph1 = ps.tile([128, chunk], f32, tag="ph1")
                        ph3 = ps.tile([128, chunk], f32, tag="ph3")
                        for dc in range(DC):
                            nc.tensor.matmul(
                                ph1[:],
                                w1_sb[:, dc, fl * 128:(fl + 1) * 128],
                                xT[:, dc, t0:t0 + chunk],
                                start=(dc == 0), stop=(dc == DC - 1),
                            )
                        for dc in range(DC):
                            nc.tensor.matmul(
                                ph3[:],
                                w3_sb[:, dc, fl * 128:(fl + 1) * 128],
                                xT[:, dc, t0:t0 + chunk],
                                start=(dc == 0), stop=(dc == DC - 1),
                            )
                        sil = silp.tile([128, chunk], bf16, tag="sil")
                        nc.scalar.activation(
                            sil[:], ph1[:], mybir.ActivationFunctionType.Silu
                        )
                        nc.vector.tensor_mul(hT[:, ft, :], sil[:], ph3[:])

                # ---- phase B: y[t, d] for this chunk ----
                for tsub in range(n_subs):
                    j = t * n_subs + tsub
                    ypsl = [
                        yps.tile([128, 512], f32, tag="yp", name=f"yp{j}_{do}")
                        for do in range(DO)
                    ]
                    for f in range(FT):
                        for do in range(DO):
                            nc.tensor.matmul(
                                ypsl[do][:],
                                hT[:, f, tsub * 128:(tsub + 1) * 128],
                                w2[:, f, do * 512:(do + 1) * 512],
                                start=(f == 0), stop=(f == FT - 1),
                            )
                    for do in range(DO):
                        ysb = yop.tile([128, 512], f32, tag="ysb")
                        nc.vector.tensor_scalar_mul(
                            ysb[:], ypsl[do][:], s_sb[:, j:j + 1]
                        )
                        nc.sync.dma_start(
                            y_d[j * 128:(j + 1) * 128, do * 512:(do + 1) * 512],
                            ysb[:],
                        )
    nc.compile()
    return nc


def _get_nc(C, chunk):
    key = (C, chunk)
    if key not in _nc_cache:
        _nc_cache[key] = _build(C, chunk)
    return _nc_cache[key]


def _capacity(max_load):
    """Pick capacity C (multiple of 128) and chunk (<=512, multiple of 128)."""
    n = max(1, -(-max_load // 512))  # ceil
    chunk = -(-max_load // (n * 128)) * 128
    return chunk * n, chunk


# ---------------------------------------------------------------- kernel ----
def kernel(hidden_states, gate_w, gate_b, w1, w3, w2, _trace=False):
    from concourse.bass_utils import run_bass_kernel_spmd

    B, S, D = hidden_states.shape
    T = B * S
    x = np.asarray(hidden_states, np.float32).reshape(T, D)
    idxs, wts = _route(x, np.asarray(gate_w, np.float32),
                       np.asarray(gate_b, np.float32))
    C, chunk = _capacity(max(len(i) for i in idxs))
    nc = _get_nc(C, chunk)

    w1 = np.asarray(w1)
    w3 = np.asarray(w3)
    w2 = np.asarray(w2)
    in_maps = []
    for e in range(NUM_EXPERTS):
        tok, wt = idxs[e], wts[e]
        l = len(tok)
        xT = np.zeros((D, C), _BF16)
        xT[:, :l] = x[tok].T.astype(_BF16)
        s = np.zeros((C,), np.float32)
        s[:l] = wt
        in_maps.append({
            "xT": xT,
            "w1": np.ascontiguousarray(w1[e]).astype(_BF16),
            "w3": np.ascontiguousarray(w3[e]).astype(_BF16),
            "w2": np.ascontiguousarray(w2[e]).astype(_BF16),
            "s": s,
        })

    res = run_bass_kernel_spmd(nc, in_maps, core_ids=list(range(N_CORES)),
                               trace=_trace)

    out = np.zeros((T, D), np.float32)
    for e in range(NUM_EXPERTS):
        tok = idxs[e]
        out[tok] += res.results[e]["y"][: len(tok)]
    out = out.reshape(B, S, D)
    if _trace:
        return out, res
    return out


# revision 9
# speedup vs baseline: 1.0107x; 1.0107x over previous
"""Trainium2 Bass kernel for nn_BlockSparseMoE (top-2 of 8 experts, SwiGLU).

Strategy (expert-parallel, sparse dispatch):
  - Host: compute router (gate matmul + softmax + top-2 + renorm) in fp64,
    gather each expert's tokens into a capacity-padded batch (the
    "all-to-all dispatch by selected expert" happens at input-sharding
    time, which is host-side by construction).
  - Device (SPMD x8, one expert per core): xT [D, C] bf16 streams through
    w1/w3 (SwiGLU) and w2 in bf16 with fp32 PSUM accumulation, rows scaled
    by the renormalized top-2 weight. No collectives needed.
  - Host: scatter-add the two expert contributions per token.

Layout per core:
  phase A: hT[f, t] = silu(x@w1)^T * (x@w3)^T computed directly transposed
           (lhsT = w1 d-chunk [128, 128f], rhs = xT d-chunk [128, tchunk])
           so no on-device transposes are ever needed.
  phase B: y[t, d] accumulated over 32 f-chunks (lhsT = hT f-chunk, rhs =
           w2 f-chunk), scaled by combine weight via per-partition scalar.
"""

import numpy as np
import ml_dtypes

HIDDEN = 1024
FFN = 4096
NUM_EXPERTS = 8
TOP_K = 2
N_CORES = 8

_BF16 = ml_dtypes.bfloat16
_nc_cache = {}


# ---------------------------------------------------------------- router ----
def _route(x, gate_w, gate_b):
    """Top-2 routing. Returns per-expert (token_idx, renorm_weight)."""
    logits = x.astype(np.float64) @ gate_w.astype(np.float64) + gate_b.astype(
        np.float64
    )
    logits -= logits.max(axis=-1, keepdims=True)
    p = np.exp(logits)
    p /= p.sum(axis=-1, keepdims=True)
    # top-2 by prob, ties broken by lower index (matches jax.lax.top_k)
    top2 = np.argsort(-p, axis=-1, kind="stable")[:, :TOP_K]
    pt = np.take_along_axis(p, top2, axis=-1)
    wt = pt / pt.sum(axis=-1, keepdims=True)
    idxs, wts = [], []
    for e in range(NUM_EXPERTS):
        mask = top2 == e  # [T, 2]
        tok = np.nonzero(mask.any(axis=-1))[0]
        w = wt[tok, np.argmax(mask[tok], axis=-1)]
        idxs.append(tok)
        wts.append(w.astype(np.float32))
    return idxs, wts


# ------------------------------------------------------------- device IR ----
def _build(C, chunk):
    """Build the per-core Bacc graph for capacity C (= chunk * n_chunks)."""
    import concourse.bacc as bacc
    import concourse.bass as bass
    import concourse.mybir as mybir
    import concourse.tile as tile

    n_chunks = C // chunk
    n_subs = chunk // 128  # 128-token sub-tiles per chunk
    DC = HIDDEN // 128  # 8 contraction chunks for x@w1
    FT = FFN // 128  # 32 f-tiles
    FG = FFN // 512  # 8 f-groups (512 wide)
    DO = HIDDEN // 512  # 2 output-d chunks

    bf16 = mybir.dt.bfloat16
    f32 = mybir.dt.float32

    nc = bacc.Bacc("TRN2", target_bir_lowering=False, debug=False,
                   num_devices=N_CORES)

    xT_d = nc.dram_tensor("xT", [HIDDEN, C], bf16, kind="ExternalInput")
    w1_d = nc.dram_tensor("w1", [HIDDEN, FFN], bf16, kind="ExternalInput")
    w3_d = nc.dram_tensor("w3", [HIDDEN, FFN], bf16, kind="ExternalInput")
    w2_d = nc.dram_tensor("w2", [FFN, HIDDEN], bf16, kind="ExternalInput")
    s_d = nc.dram_tensor("s", [C], f32, kind="ExternalInput")
    y_d = nc.dram_tensor("y", [C, HIDDEN], f32, kind="ExternalOutput")

    # DRAM views tiled for 128-partition DMA
    xT_v = xT_d.ap().rearrange("(dc p) c -> p dc c", p=128)
    w1_v = w1_d.ap().rearrange("(dc p) f -> p dc f", p=128)
    w3_v = w3_d.ap().rearrange("(dc p) f -> p dc f", p=128)
    w2_v = w2_d.ap().rearrange("(ft p) d -> p ft d", p=128)
    s_v = s_d.ap().rearrange("(j p) -> p j", p=128)

    with tile.TileContext(nc) as tc:
        with (
            tc.tile_pool(name="res", bufs=1) as res,
            tc.tile_pool(name="w13", bufs=2) as w13,
            tc.tile_pool(name="hp", bufs=2) as hp,
            tc.tile_pool(name="sil", bufs=4) as silp,
            tc.tile_pool(name="yo", bufs=4) as yop,
            tc.tile_pool(name="ps", bufs=2, space=bass.MemorySpace.PSUM) as ps,
            tc.tile_pool(name="yps", bufs=4, space=bass.MemorySpace.PSUM) as yps,
        ):
            # resident tensors.  DMA order matters: the PE's first work is
            # chunk-0 phase A, which needs xT[:, :, :chunk] and w1/w3 of
            # fg 0 — so those transfers go first; the 8MB w2 (needed only
            # for phase B, ~85us in) is interleaved across chunk-0's
            # f-group loop so it never blocks the critical path.
            xT = res.tile([128, DC, C], bf16, tag="xT")
            w2 = res.tile([128, FT, HIDDEN], bf16, tag="w2")
            s_sb = res.tile([128, C // 128], f32, tag="s")
            nc.sync.dma_start(xT[:, :, 0:chunk], xT_v[:, :, 0:chunk])

            for t in range(n_chunks):
                t0 = t * chunk
                hT = hp.tile([128, FT, chunk], bf16, tag="hT")
                # ---- phase A: hT[f, t] for this token chunk ----
                for fg in range(FG):
                    w1_sb = w13.tile([128, DC, 512], bf16, tag="w1")
                    w3_sb = w13.tile([128, DC, 512], bf16, tag="w3")
                    fsl = slice(fg * 512, (fg + 1) * 512)
                    nc.sync.dma_start(w1_sb[:], w1_v[:, :, fsl])
                    nc.sync.dma_start(w3_sb[:], w3_v[:, :, fsl])
                    if t == 0 and fg >= 1:
                        # stream w2 (1MB slices) strictly behind the
                        # critical early w1/w3 loads; all slices land
                        # before phase B needs them (~85us in)
                        for q in ([fg - 1] if fg < FG - 1 else [FG - 2, FG - 1]):
                            nc.sync.dma_start(w2[:, q * 4:(q + 1) * 4, :],
                                              w2_v[:, q * 4:(q + 1) * 4, :])
                        if fg == 6:
                            nc.sync.dma_start(s_sb[:], s_v)
                        if fg in (3, 5):
                            # prefetch later chunks' xT slices
                            tn = {3: 1, 5: 2}[fg]
                            if tn < n_chunks:
                                nc.sync.dma_start(
                                    xT[:, :, tn * chunk:(tn + 1) * chunk],
                                    xT_v[:, :, tn * chunk:(tn + 1) * chunk],
                                )
                    for fl in range(4):
                        ft = fg * 4 + fl
                        ph1 = ps.tile([128, chunk], f32, tag="ph1")
                        ph3 = ps.tile([128, chunk], f32, tag="ph3")
                        for dc in range(DC):
                            nc.tensor.matmul(
                                ph1[:],
                                w1_sb[:, dc, fl * 128:(fl + 1) * 128],
                                xT[:, dc, t0:t0 + chunk],
                                start=(dc == 0), stop=(dc == DC - 1),
                            )
                        for dc in range(DC):
                            nc.tensor.matmul(
                                ph3[:],
                                w3_sb[:, dc, fl * 128:(fl + 1) * 128],
                                xT[:, dc, t0:t0 + chunk],
                                start=(dc == 0), stop=(dc == DC - 1),
                            )
                        sil = silp.tile([128, chunk], bf16, tag="sil")
                        nc.scalar.activation(
                            sil[:], ph1[:], mybir.ActivationFunctionType.Silu
                        )
                        nc.vector.tensor_mul(hT[:, ft, :], sil[:], ph3[:])

                # ---- phase B: y[t, d] for this chunk ----
                for tsub in range(n_subs):
                    j = t * n_subs + tsub
                    ypsl = [
                        yps.tile([128, 512], f32, tag="yp", name=f"yp{j}_{do}")
                        for do in range(DO)
                    ]
                    for f in range(FT):
                        for do in range(DO):
                            nc.tensor.matmul(
                                ypsl[do][:],
                                hT[:, f, tsub * 128:(tsub + 1) * 128],
                                w2[:, f, do * 512:(do + 1) * 512],
                                start=(f == 0), stop=(f == FT - 1),
                            )
                    for do in range(DO):
                        ysb = yop.tile([128, 512], f32, tag="ysb")
                        # scale on ScalarE (idle at phase-B tail; DVE is
                        # still draining phase-A muls): out = in * s
                        nc.scalar.activation(
                            ysb[:], ypsl[do][:],
                            mybir.ActivationFunctionType.Copy,
                            scale=s_sb[:, j:j + 1],
                        )
                        nc.sync.dma_start(
                            y_d[j * 128:(j + 1) * 128, do * 512:(do + 1) * 512],
                            ysb[:],
                        )
    nc.compile()
    return nc


def _get_nc(C, chunk):
    key = (C, chunk)
    if key not in _nc_cache:
        _nc_cache[key] = _build(C, chunk)
    return _nc_cache[key]


def _capacity(max_load):
    """Pick capacity C (multiple of 128) and chunk (<=512, multiple of 128)."""
    n = max(1, -(-max_load // 512))  # ceil
    chunk = -(-max_load // (n * 128)) * 128
    return chunk * n, chunk


# ---------------------------------------------------------------- kernel ----
def kernel(hidden_states, gate_w, gate_b, w1, w3, w2, _trace=False):
    from concourse.bass_utils import run_bass_kernel_spmd

    B, S, D = hidden_states.shape
    T = B * S
    x = np.asarray(hidden_states, np.float32).reshape(T, D)
    idxs, wts = _route(x, np.asarray(gate_w, np.float32),
                       np.asarray(gate_b, np.float32))
    C, chunk = _capacity(max(len(i) for i in idxs))
    nc = _get_nc(C, chunk)

    w1 = np.asarray(w1)
    w3 = np.asarray(w3)
    w2 = np.asarray(w2)
    in_maps = []
    for e in range(NUM_EXPERTS):
        tok, wt = idxs[e], wts[e]
        l = len(tok)
        xT = np.zeros((D, C), _BF16)
        xT[:, :l] = x[tok].T.astype(_BF16)
        s = np.zeros((C,), np.float32)
        s[:l] = wt
        in_maps.append({
            "xT": xT,
            "w1": np.ascontiguousarray(w1[e]).astype(_BF16),
            "w3": np.ascontiguousarray(w3[e]).astype(_BF16),
            "w2": np.ascontiguousarray(w2[e]).astype(_BF16),
            "s": s,
        })

    res = run_bass_kernel_spmd(nc, in_maps, core_ids=list(range(N_CORES)),
                               trace=_trace)

    out = np.zeros((T, D), np.float32)
    for e in range(NUM_EXPERTS):
        tok = idxs[e]
        out[tok] += res.results[e]["y"][: len(tok)]
    out = out.reshape(B, S, D)
    if _trace:
        return out, res
    return out


# revision 13
# speedup vs baseline: 1.0127x; 1.0021x over previous
"""Trainium2 Bass kernel for nn_BlockSparseMoE (top-2 of 8 experts, SwiGLU).

Strategy (expert-parallel, sparse dispatch):
  - Host: compute router (gate matmul + softmax + top-2 + renorm) in fp64,
    gather each expert's tokens into a capacity-padded batch (the
    "all-to-all dispatch by selected expert" happens at input-sharding
    time, which is host-side by construction).
  - Device (SPMD x8, one expert per core): xT [D, C] bf16 streams through
    w1/w3 (SwiGLU) and w2 in bf16 with fp32 PSUM accumulation, rows scaled
    by the renormalized top-2 weight. No collectives needed.
  - Host: scatter-add the two expert contributions per token.

Layout per core:
  phase A: hT[f, t] = silu(x@w1)^T * (x@w3)^T computed directly transposed
           (lhsT = w1 d-chunk [128, 128f], rhs = xT d-chunk [128, tchunk])
           so no on-device transposes are ever needed.
  phase B: y[t, d] accumulated over 32 f-chunks (lhsT = hT f-chunk, rhs =
           w2 f-chunk), scaled by combine weight via per-partition scalar.
"""

import numpy as np
import ml_dtypes

HIDDEN = 1024
FFN = 4096
NUM_EXPERTS = 8
TOP_K = 2
N_CORES = 8

_BF16 = ml_dtypes.bfloat16
_nc_cache = {}


# ---------------------------------------------------------------- router ----
def _route(x, gate_w, gate_b):
    """Top-2 routing. Returns per-expert (token_idx, renorm_weight)."""
    logits = x.astype(np.float64) @ gate_w.astype(np.float64) + gate_b.astype(
        np.float64
    )
    logits -= logits.max(axis=-1, keepdims=True)
    p = np.exp(logits)
    p /= p.sum(axis=-1, keepdims=True)
    # top-2 by prob, ties broken by lower index (matches jax.lax.top_k)
    top2 = np.argsort(-p, axis=-1, kind="stable")[:, :TOP_K]
    pt = np.take_along_axis(p, top2, axis=-1)
    wt = pt / pt.sum(axis=-1, keepdims=True)
    idxs, wts = [], []
    for e in range(NUM_EXPERTS):
        mask = top2 == e  # [T, 2]
        tok = np.nonzero(mask.any(axis=-1))[0]
        w = wt[tok, np.argmax(mask[tok], axis=-1)]
        idxs.append(tok)
        wts.append(w.astype(np.float32))
    return idxs, wts


# ------------------------------------------------------------- device IR ----
def _build(C, chunk):
    """Build the per-core Bacc graph for capacity C (= chunk * n_chunks)."""
    import concourse.bacc as bacc
    import concourse.bass as bass
    import concourse.mybir as mybir
    import concourse.tile as tile

    n_chunks = C // chunk
    n_subs = chunk // 128  # 128-token sub-tiles per chunk
    DC = HIDDEN // 128  # 8 contraction chunks for x@w1
    FT = FFN // 128  # 32 f-tiles
    FG = FFN // 512  # 8 f-groups (512 wide)
    DO = HIDDEN // 512  # 2 output-d chunks

    bf16 = mybir.dt.bfloat16
    f32 = mybir.dt.float32

    nc = bacc.Bacc("TRN2", target_bir_lowering=False, debug=False,
                   num_devices=N_CORES)

    xT_d = nc.dram_tensor("xT", [HIDDEN, C], bf16, kind="ExternalInput")
    # w1/w3 arrive host-pre-tiled as [fg, p, dc, 512] so each f-group DMA
    # is one fully-contiguous 8KB line per partition (vs 1KB strided)
    w1_d = nc.dram_tensor("w1", [FG, 128, DC, 512], bf16, kind="ExternalInput")
    w3_d = nc.dram_tensor("w3", [FG, 128, DC, 512], bf16, kind="ExternalInput")
    w2_d = nc.dram_tensor("w2", [FFN, HIDDEN], bf16, kind="ExternalInput")
    s_d = nc.dram_tensor("s", [C], f32, kind="ExternalInput")
    y_d = nc.dram_tensor("y", [C, HIDDEN], f32, kind="ExternalOutput")

    # DRAM views tiled for 128-partition DMA
    xT_v = xT_d.ap().rearrange("(dc p) c -> p dc c", p=128)
    w2_v = w2_d.ap().rearrange("(ft p) d -> p ft d", p=128)
    s_v = s_d.ap().rearrange("(j p) -> p j", p=128)

    with tile.TileContext(nc) as tc:
        with (
            tc.tile_pool(name="res", bufs=1) as res,
            tc.tile_pool(name="w13", bufs=2) as w13,
            tc.tile_pool(name="hp", bufs=2) as hp,
            tc.tile_pool(name="sil", bufs=4) as silp,
            tc.tile_pool(name="yo", bufs=4) as yop,
            tc.tile_pool(name="ps", bufs=2, space=bass.MemorySpace.PSUM) as ps,
            tc.tile_pool(name="yps", bufs=4, space=bass.MemorySpace.PSUM) as yps,
        ):
            # resident tensors.  DMA order matters: the PE's first work is
            # chunk-0 phase A, which needs xT[:, :, :chunk] and w1/w3 of
            # fg 0 — so those transfers go first; the 8MB w2 (needed only
            # for phase B, ~85us in) is interleaved across chunk-0's
            # f-group loop so it never blocks the critical path.
            xT = res.tile([128, DC, C], bf16, tag="xT")
            w2 = res.tile([128, FT, HIDDEN], bf16, tag="w2")
            s_sb = res.tile([128, C // 128], f32, tag="s")
            nc.sync.dma_start(xT[:, :, 0:chunk], xT_v[:, :, 0:chunk])

            for t in range(n_chunks):
                t0 = t * chunk
                hT = hp.tile([128, FT, chunk], bf16, tag="hT")
                # ---- phase A: hT[f, t] for this token chunk ----
                for fg in range(FG):
                    w1_sb = w13.tile([128, DC, 512], bf16, tag="w1")
                    w3_sb = w13.tile([128, DC, 512], bf16, tag="w3")
                    nc.sync.dma_start(w1_sb[:], w1_d.ap()[fg])
                    nc.sync.dma_start(w3_sb[:], w3_d.ap()[fg])
                    if t == 0 and fg >= 1:
                        # stream w2 (1MB slices) strictly behind the
                        # critical early w1/w3 loads; all slices land
                        # before phase B needs them (~85us in)
                        for q in ([fg - 1] if fg < FG - 1 else [FG - 2, FG - 1]):
                            nc.sync.dma_start(w2[:, q * 4:(q + 1) * 4, :],
                                              w2_v[:, q * 4:(q + 1) * 4, :])
                        if fg == 6:
                            nc.sync.dma_start(s_sb[:], s_v)
                        if fg in (3, 5):
                            # prefetch later chunks' xT slices
                            tn = {3: 1, 5: 2}[fg]
                            if tn < n_chunks:
                                nc.sync.dma_start(
                                    xT[:, :, tn * chunk:(tn + 1) * chunk],
                                    xT_v[:, :, tn * chunk:(tn + 1) * chunk],
                                )
                    for fl in range(4):
                        ft = fg * 4 + fl
                        ph1 = ps.tile([128, chunk], f32, tag="ph1")
                        ph3 = ps.tile([128, chunk], f32, tag="ph3")
                        for dc in range(DC):
                            nc.tensor.matmul(
                                ph1[:],
                                w1_sb[:, dc, fl * 128:(fl + 1) * 128],
                                xT[:, dc, t0:t0 + chunk],
                                start=(dc == 0), stop=(dc == DC - 1),
                            )
                        for dc in range(DC):
                            nc.tensor.matmul(
                                ph3[:],
                                w3_sb[:, dc, fl * 128:(fl + 1) * 128],
                                xT[:, dc, t0:t0 + chunk],
                                start=(dc == 0), stop=(dc == DC - 1),
                            )
                        sil = silp.tile([128, chunk], bf16, tag="sil")
                        nc.scalar.activation(
                            sil[:], ph1[:], mybir.ActivationFunctionType.Silu
                        )
                        nc.vector.tensor_mul(hT[:, ft, :], sil[:], ph3[:])

                # ---- phase B: y[t, d] for this chunk ----
                for tsub in range(n_subs):
                    j = t * n_subs + tsub
                    ypsl = [
                        yps.tile([128, 512], f32, tag="yp", name=f"yp{j}_{do}")
                        for do in range(DO)
                    ]
                    for f in range(FT):
                        for do in range(DO):
                            nc.tensor.matmul(
                                ypsl[do][:],
                                hT[:, f, tsub * 128:(tsub + 1) * 128],
                                w2[:, f, do * 512:(do + 1) * 512],
                                start=(f == 0), stop=(f == FT - 1),
                            )
                    for do in range(DO):
                        ysb = yop.tile([128, 512], f32, tag="ysb")
                        # scale on ScalarE (idle at phase-B tail; DVE is
                        # still draining phase-A muls): out = in * s
                        nc.scalar.activation(
                            ysb[:], ypsl[do][:],
                            mybir.ActivationFunctionType.Copy,
                            scale=s_sb[:, j:j + 1],
                        )
                        nc.sync.dma_start(
                            y_d[j * 128:(j + 1) * 128, do * 512:(do + 1) * 512],
                            ysb[:],
                        )
    nc.compile()
    return nc


def _get_nc(C, chunk):
    key = (C, chunk)
    if key not in _nc_cache:
        _nc_cache[key] = _build(C, chunk)
    return _nc_cache[key]


def _capacity(max_load):
    """Pick capacity C (multiple of 128) and chunk (<=512, multiple of 128)."""
    n = max(1, -(-max_load // 512))  # ceil
    chunk = -(-max_load // (n * 128)) * 128
    return chunk * n, chunk


def _pretile_w13(w):
    """[HIDDEN, FFN] -> [fg, p, dc, 512] bf16 (see _build's w1_d layout)."""
    w4 = np.ascontiguousarray(w).reshape(HIDDEN // 128, 128, FFN // 512, 512)
    return np.ascontiguousarray(w4.transpose(2, 1, 0, 3)).astype(_BF16)


# ---------------------------------------------------------------- kernel ----
def kernel(hidden_states, gate_w, gate_b, w1, w3, w2, _trace=False):
    from concourse.bass_utils import run_bass_kernel_spmd

    B, S, D = hidden_states.shape
    T = B * S
    x = np.asarray(hidden_states, np.float32).reshape(T, D)
    idxs, wts = _route(x, np.asarray(gate_w, np.float32),
                       np.asarray(gate_b, np.float32))
    C, chunk = _capacity(max(len(i) for i in idxs))
    nc = _get_nc(C, chunk)

    w1 = np.asarray(w1)
    w3 = np.asarray(w3)
    w2 = np.asarray(w2)
    in_maps = []
    for e in range(NUM_EXPERTS):
        tok, wt = idxs[e], wts[e]
        l = len(tok)
        xT = np.zeros((D, C), _BF16)
        xT[:, :l] = x[tok].T.astype(_BF16)
        s = np.zeros((C,), np.float32)
        s[:l] = wt
        in_maps.append({
            "xT": xT,
            "w1": _pretile_w13(w1[e]),
            "w3": _pretile_w13(w3[e]),
            "w2": np.ascontiguousarray(w2[e]).astype(_BF16),
            "s": s,
        })

    res = run_bass_kernel_spmd(nc, in_maps, core_ids=list(range(N_CORES)),
                               trace=_trace)

    out = np.zeros((T, D), np.float32)
    for e in range(NUM_EXPERTS):
        tok = idxs[e]
        out[tok] += res.results[e]["y"][: len(tok)]
    out = out.reshape(B, S, D)
    if _trace:
        return out, res
    return out


# revision 22
# speedup vs baseline: 1.0368x; 1.0237x over previous
"""Trainium2 Bass kernel for nn_BlockSparseMoE (top-2 of 8 experts, SwiGLU).

Strategy (expert-parallel, sparse dispatch):
  - Host: compute router (gate matmul + softmax + top-2 + renorm) in fp64,
    gather each expert's tokens into a capacity-padded batch (the
    "all-to-all dispatch by selected expert" happens at input-sharding
    time, which is host-side by construction).
  - Device (SPMD x8, one expert per core): xT [D, C] bf16 streams through
    w1/w3 (SwiGLU) and w2 in bf16 with fp32 PSUM accumulation, rows scaled
    by the renormalized top-2 weight. No collectives needed.
  - Host: scatter-add the two expert contributions per token.

Layout per core:
  phase A: hT[f, t] = silu(x@w1)^T * (x@w3)^T computed directly transposed
           (lhsT = w1 d-chunk [128, 128f], rhs = xT d-chunk [128, tchunk])
           so no on-device transposes are ever needed.
  phase B: y[t, d] accumulated over 32 f-chunks (lhsT = hT f-chunk, rhs =
           w2 f-chunk), scaled by combine weight via per-partition scalar.
"""

import numpy as np
import ml_dtypes

HIDDEN = 1024
FFN = 4096
NUM_EXPERTS = 8
TOP_K = 2
N_CORES = 8

_BF16 = ml_dtypes.bfloat16
_nc_cache = {}


# ---------------------------------------------------------------- router ----
def _route(x, gate_w, gate_b):
    """Top-2 routing. Returns per-expert (token_idx, renorm_weight)."""
    logits = x.astype(np.float64) @ gate_w.astype(np.float64) + gate_b.astype(
        np.float64
    )
    logits -= logits.max(axis=-1, keepdims=True)
    p = np.exp(logits)
    p /= p.sum(axis=-1, keepdims=True)
    # top-2 by prob, ties broken by lower index (matches jax.lax.top_k)
    top2 = np.argsort(-p, axis=-1, kind="stable")[:, :TOP_K]
    pt = np.take_along_axis(p, top2, axis=-1)
    wt = pt / pt.sum(axis=-1, keepdims=True)
    idxs, wts = [], []
    for e in range(NUM_EXPERTS):
        mask = top2 == e  # [T, 2]
        tok = np.nonzero(mask.any(axis=-1))[0]
        w = wt[tok, np.argmax(mask[tok], axis=-1)]
        idxs.append(tok)
        wts.append(w.astype(np.float32))
    return idxs, wts


# ------------------------------------------------------------- device IR ----
def _build(C, chunks):
    """Build the per-core Bacc graph for capacity C (= sum(chunks)).

    All chunks except the last are multiples of 128 (so combine-weight
    columns stay globally 128-aligned); the last may be any multiple of
    16 and its final phase-B sub-tile may have <128 partitions.
    """
    import concourse.bacc as bacc
    import concourse.bass as bass
    import concourse.mybir as mybir
    import concourse.tile as tile

    n_chunks = len(chunks)
    DC = HIDDEN // 128  # 8 contraction chunks for x@w1
    FT = FFN // 128  # 32 f-tiles
    FG = FFN // 512  # 8 f-groups (512 wide)
    DO = HIDDEN // 512  # 2 output-d chunks
    S_PAD = -(-C // 128) * 128  # s input padded to whole 128-columns

    bf16 = mybir.dt.bfloat16
    f32 = mybir.dt.float32

    nc = bacc.Bacc("TRN2", target_bir_lowering=False, debug=False,
                   num_devices=N_CORES)

    xT_d = nc.dram_tensor("xT", [HIDDEN, C], bf16, kind="ExternalInput")
    # w1/w3 arrive host-pre-tiled as [fg, p, dc, 512] so each f-group DMA
    # is one fully-contiguous 8KB line per partition (vs 1KB strided)
    w1_d = nc.dram_tensor("w1", [FG, 128, DC, 512], bf16, kind="ExternalInput")
    w3_d = nc.dram_tensor("w3", [FG, 128, DC, 512], bf16, kind="ExternalInput")
    w2_d = nc.dram_tensor("w2", [FFN, HIDDEN], bf16, kind="ExternalInput")
    s_d = nc.dram_tensor("s", [S_PAD], f32, kind="ExternalInput")
    y_d = nc.dram_tensor("y", [C, HIDDEN], f32, kind="ExternalOutput")

    # DRAM views tiled for 128-partition DMA
    xT_v = xT_d.ap().rearrange("(dc p) c -> p dc c", p=128)
    w2_v = w2_d.ap().rearrange("(ft p) d -> p ft d", p=128)
    s_v = s_d.ap().rearrange("(j p) -> p j", p=128)

    with tile.TileContext(nc) as tc:
        with (
            tc.tile_pool(name="res", bufs=1) as res,
            tc.tile_pool(name="w13", bufs=2) as w13,
            tc.tile_pool(name="hp", bufs=2) as hp,
            tc.tile_pool(name="sil", bufs=4) as silp,
            tc.tile_pool(name="yo", bufs=4) as yop,
            tc.tile_pool(name="ps", bufs=2, space=bass.MemorySpace.PSUM) as ps,
            tc.tile_pool(name="yps", bufs=4, space=bass.MemorySpace.PSUM) as yps,
        ):
            # resident tensors.  DMA order matters: the PE's first work is
            # chunk-0 phase A, which needs xT[:, :, :chunk] and w1/w3 of
            # fg 0 — so those transfers go first; the 8MB w2 (needed only
            # for phase B, ~85us in) is interleaved across chunk-0's
            # f-group loop so it never blocks the critical path.
            xT = res.tile([128, DC, C], bf16, tag="xT")
            w2 = res.tile([128, FT, HIDDEN], bf16, tag="w2")
            s_sb = res.tile([128, S_PAD // 128], f32, tag="s")

            # HAM pre-warm: the first ~13us are DMA-bound (ring priming +
            # the first weight tiles), during which the PE would idle and
            # its clock gate stays at 1.2GHz.  A burst of throwaway
            # matmuls on a zeroed scratch tile keeps the activity monitor
            # busy so the real matmuls start at the full 2.4GHz.
            warm_sb = silp.tile([128, 128], bf16, tag="warm_in", bufs=1)
            nc.gpsimd.memset(warm_sb[:], 0.0)
            warm_ps = ps.tile([128, 128], f32, tag="ph1", name="warm_ps")
            N_WARM = 160
            for i in range(N_WARM):
                nc.tensor.matmul(warm_ps[:], warm_sb[:], warm_sb[:],
                                 start=(i == 0), stop=(i == N_WARM - 1))

            nc.sync.dma_start(xT[:, :, 0:chunks[0]], xT_v[:, :, 0:chunks[0]])

            t0 = 0
            for t, chunk in enumerate(chunks):
                hT = hp.tile([128, FT, chunk], bf16, tag="hT")
                # ---- phase A: hT[f, t] for this token chunk ----
                for fg in range(FG):
                    if t == 0 and fg == 0:
                        # first weight tiles split in dc-halves (separate
                        # tiles: Tile deps are tile-granular) so the
                        # first accumulation starts after 1.75MB, not
                        # 2.75MB, of startup DMA
                        w1a = w13.tile([128, 4, 512], bf16, tag="w1a", bufs=1)
                        w3a = w13.tile([128, 4, 512], bf16, tag="w3a", bufs=1)
                        w1b = w13.tile([128, 4, 512], bf16, tag="w1b", bufs=1)
                        w3b = w13.tile([128, 4, 512], bf16, tag="w3b", bufs=1)
                        nc.sync.dma_start(w1a[:], w1_d.ap()[0][:, 0:4, :])
                        nc.sync.dma_start(w3a[:], w3_d.ap()[0][:, 0:4, :])
                        nc.sync.dma_start(w1b[:], w1_d.ap()[0][:, 4:DC, :])
                        nc.sync.dma_start(w3b[:], w3_d.ap()[0][:, 4:DC, :])
                        w1_parts = [(w1a, 0), (w1b, 4)]
                        w3_parts = [(w3a, 0), (w3b, 4)]
                    else:
                        w1_sb = w13.tile([128, DC, 512], bf16, tag="w1")
                        w3_sb = w13.tile([128, DC, 512], bf16, tag="w3")
                        nc.sync.dma_start(w1_sb[:], w1_d.ap()[fg])
                        nc.sync.dma_start(w3_sb[:], w3_d.ap()[fg])
                        w1_parts = [(w1_sb, 0)]
                        w3_parts = [(w3_sb, 0)]
                    if t == 0 and fg >= 1:
                        # stream w2 (1MB slices) strictly behind the
                        # critical early w1/w3 loads; all slices land
                        # before phase B needs them (~85us in)
                        for q in ([fg - 1] if fg < FG - 1 else [FG - 2, FG - 1]):
                            nc.sync.dma_start(w2[:, q * 4:(q + 1) * 4, :],
                                              w2_v[:, q * 4:(q + 1) * 4, :])
                        if fg == 6:
                            nc.sync.dma_start(s_sb[:], s_v)
                        if fg in (3, 5):
                            # prefetch the next two chunks' xT slices
                            tn = {3: 1, 5: 2}[fg]
                            if tn < n_chunks:
                                o = sum(chunks[:tn])
                                nc.sync.dma_start(
                                    xT[:, :, o:o + chunks[tn]],
                                    xT_v[:, :, o:o + chunks[tn]],
                                )
                    if t >= 1 and fg == 1 and t + 2 < n_chunks:
                        # chunks 3+ prefetched one-ahead from chunk t>=1
                        tn = t + 2
                        o = sum(chunks[:tn])
                        nc.sync.dma_start(
                            xT[:, :, o:o + chunks[tn]],
                            xT_v[:, :, o:o + chunks[tn]],
                        )

                    def _wslice(parts, dc):
                        for tile_, base in parts:
                            if base <= dc < base + tile_.shape[1]:
                                return tile_[:, dc - base, :]
                        raise AssertionError(dc)

                    for fl in range(4):
                        ft = fg * 4 + fl
                        ph1 = ps.tile([128, chunk], f32, tag="ph1")
                        ph3 = ps.tile([128, chunk], f32, tag="ph3")
                        for dc in range(DC):
                            nc.tensor.matmul(
                                ph1[:],
                                _wslice(w1_parts, dc)[:, fl * 128:(fl + 1) * 128],
                                xT[:, dc, t0:t0 + chunk],
                                start=(dc == 0), stop=(dc == DC - 1),
                            )
                        for dc in range(DC):
                            nc.tensor.matmul(
                                ph3[:],
                                _wslice(w3_parts, dc)[:, fl * 128:(fl + 1) * 128],
                                xT[:, dc, t0:t0 + chunk],
                                start=(dc == 0), stop=(dc == DC - 1),
                            )
                        sil = silp.tile([128, chunk], bf16, tag="sil")
                        nc.scalar.activation(
                            sil[:], ph1[:], mybir.ActivationFunctionType.Silu
                        )
                        nc.vector.tensor_mul(hT[:, ft, :], sil[:], ph3[:])

                # ---- phase B: y[t, d] for this chunk ----
                # do-outer so do=0's scale+store overlaps do=1's matmuls
                # (shortens the kernel-exit tail after the last matmul)
                subs = [128] * (chunk // 128)
                if chunk % 128:
                    subs.append(chunk % 128)
                for tsub, tsz in enumerate(subs):
                    j = (t0 + tsub * 128) // 128
                    o = tsub * 128
                    for do in range(DO):
                        yp = yps.tile([128, 512], f32, tag="yp",
                                      name=f"yp{j}_{do}")
                        for f in range(FT):
                            nc.tensor.matmul(
                                yp[0:tsz, :],
                                hT[:, f, o:o + tsz],
                                w2[:, f, do * 512:(do + 1) * 512],
                                start=(f == 0), stop=(f == FT - 1),
                            )
                        ysb = yop.tile([128, 512], f32, tag="ysb")
                        # scale on ScalarE: out = in * s (per-partition)
                        nc.scalar.activation(
                            ysb[0:tsz, :], yp[0:tsz, :],
                            mybir.ActivationFunctionType.Copy,
                            scale=s_sb[0:tsz, j:j + 1],
                        )
                        nc.sync.dma_start(
                            y_d[t0 + o:t0 + o + tsz,
                                do * 512:(do + 1) * 512],
                            ysb[0:tsz, :],
                        )
                t0 += chunk
    nc.compile()
    return nc


def _get_nc(C, chunks):
    key = (C, chunks)
    if key not in _nc_cache:
        _nc_cache[key] = _build(C, chunks)
    return _nc_cache[key]


def _capacity(max_load):
    """Capacity C (multiple of 16) split into chunks: all but the last are
    multiples of 128 in [240, 512]; the last is a multiple of 16 kept
    >=240 when possible (below ~233 tokens phase-A matmuls go
    LDWEIGHTS-bound, wasting more than the padding saves)."""
    C = -(-max_load // 16) * 16
    n = max(1, -(-C // 512))
    chunks = []
    rem = C
    for i in range(n - 1):
        c = min(512, -(-rem // ((n - i) * 128)) * 128)
        chunks.append(c)
        rem -= c
    # keep the ragged tail >= 240 by shrinking earlier 128-mult chunks
    while n > 1 and rem < 240 and chunks:
        for i in range(len(chunks)):
            if rem >= 240:
                break
            if chunks[i] > 256:
                chunks[i] -= 128
                rem += 128
        else:
            break
    chunks.append(rem)
    assert sum(chunks) == C and all(c > 0 for c in chunks)
    return C, tuple(chunks)


def _pretile_w13(w):
    """[HIDDEN, FFN] -> [fg, p, dc, 512] bf16 (see _build's w1_d layout)."""
    w4 = np.ascontiguousarray(w).reshape(HIDDEN // 128, 128, FFN // 512, 512)
    return np.ascontiguousarray(w4.transpose(2, 1, 0, 3)).astype(_BF16)


# ---------------------------------------------------------------- kernel ----
def kernel(hidden_states, gate_w, gate_b, w1, w3, w2, _trace=False):
    from concourse.bass_utils import run_bass_kernel_spmd

    B, S, D = hidden_states.shape
    T = B * S
    x = np.asarray(hidden_states, np.float32).reshape(T, D)
    idxs, wts = _route(x, np.asarray(gate_w, np.float32),
                       np.asarray(gate_b, np.float32))
    C, chunks = _capacity(max(len(i) for i in idxs))
    S_PAD = -(-C // 128) * 128
    nc = _get_nc(C, chunks)

    w1 = np.asarray(w1)
    w3 = np.asarray(w3)
    w2 = np.asarray(w2)
    in_maps = []
    for e in range(NUM_EXPERTS):
        tok, wt = idxs[e], wts[e]
        l = len(tok)
        xT = np.zeros((D, C), _BF16)
        xT[:, :l] = x[tok].T.astype(_BF16)
        s = np.zeros((S_PAD,), np.float32)
        s[:l] = wt
        in_maps.append({
            "xT": xT,
            "w1": _pretile_w13(w1[e]),
            "w3": _pretile_w13(w3[e]),
            "w2": np.ascontiguousarray(w2[e]).astype(_BF16),
            "s": s,
        })

    res = run_bass_kernel_spmd(nc, in_maps, core_ids=list(range(N_CORES)),
                               trace=_trace)

    out = np.zeros((T, D), np.float32)
    for e in range(NUM_EXPERTS):
        tok = idxs[e]
        out[tok] += res.results[e]["y"][: len(tok)]
    out = out.reshape(B, S, D)
    if _trace:
        return out, res
    return out


# revision 23
# speedup vs baseline: 1.0488x; 1.0116x over previous
"""Trainium2 Bass kernel for nn_BlockSparseMoE (top-2 of 8 experts, SwiGLU).

Strategy (expert-parallel, sparse dispatch):
  - Host: compute router (gate matmul + softmax + top-2 + renorm) in fp64,
    gather each expert's tokens into a capacity-padded batch (the
    "all-to-all dispatch by selected expert" happens at input-sharding
    time, which is host-side by construction).
  - Device (SPMD x8, one expert per core): xT [D, C] bf16 streams through
    w1/w3 (SwiGLU) and w2 in bf16 with fp32 PSUM accumulation, rows scaled
    by the renormalized top-2 weight. No collectives needed.
  - Host: scatter-add the two expert contributions per token.

Layout per core:
  phase A: hT[f, t] = silu(x@w1)^T * (x@w3)^T computed directly transposed
           (lhsT = w1 d-chunk [128, 128f], rhs = xT d-chunk [128, tchunk])
           so no on-device transposes are ever needed.
  phase B: y[t, d] accumulated over 32 f-chunks (lhsT = hT f-chunk, rhs =
           w2 f-chunk), scaled by combine weight via per-partition scalar.
"""

import numpy as np
import ml_dtypes

HIDDEN = 1024
FFN = 4096
NUM_EXPERTS = 8
TOP_K = 2
N_CORES = 8

_BF16 = ml_dtypes.bfloat16
_nc_cache = {}


# ---------------------------------------------------------------- router ----
def _route(x, gate_w, gate_b):
    """Top-2 routing. Returns per-expert (token_idx, renorm_weight)."""
    logits = x.astype(np.float64) @ gate_w.astype(np.float64) + gate_b.astype(
        np.float64
    )
    logits -= logits.max(axis=-1, keepdims=True)
    p = np.exp(logits)
    p /= p.sum(axis=-1, keepdims=True)
    # top-2 by prob, ties broken by lower index (matches jax.lax.top_k)
    top2 = np.argsort(-p, axis=-1, kind="stable")[:, :TOP_K]
    pt = np.take_along_axis(p, top2, axis=-1)
    wt = pt / pt.sum(axis=-1, keepdims=True)
    idxs, wts = [], []
    for e in range(NUM_EXPERTS):
        mask = top2 == e  # [T, 2]
        tok = np.nonzero(mask.any(axis=-1))[0]
        w = wt[tok, np.argmax(mask[tok], axis=-1)]
        idxs.append(tok)
        wts.append(w.astype(np.float32))
    return idxs, wts


# ------------------------------------------------------------- device IR ----
def _build(C, chunks):
    """Build the per-core Bacc graph for capacity C (= sum(chunks)).

    All chunks except the last are multiples of 128 (so combine-weight
    columns stay globally 128-aligned); the last may be any multiple of
    16 and its final phase-B sub-tile may have <128 partitions.
    """
    import concourse.bacc as bacc
    import concourse.bass as bass
    import concourse.mybir as mybir
    import concourse.tile as tile

    n_chunks = len(chunks)
    DC = HIDDEN // 128  # 8 contraction chunks for x@w1
    FT = FFN // 128  # 32 f-tiles
    FG = FFN // 512  # 8 f-groups (512 wide)
    DO = HIDDEN // 512  # 2 output-d chunks
    S_PAD = -(-C // 128) * 128  # s input padded to whole 128-columns

    bf16 = mybir.dt.bfloat16
    f32 = mybir.dt.float32

    nc = bacc.Bacc("TRN2", target_bir_lowering=False, debug=False,
                   num_devices=N_CORES)

    xT_d = nc.dram_tensor("xT", [HIDDEN, C], bf16, kind="ExternalInput")
    # w1/w3 arrive host-pre-tiled as [fg, p, dc, 512] so each f-group DMA
    # is one fully-contiguous 8KB line per partition (vs 1KB strided)
    w1_d = nc.dram_tensor("w1", [FG, 128, DC, 512], bf16, kind="ExternalInput")
    w3_d = nc.dram_tensor("w3", [FG, 128, DC, 512], bf16, kind="ExternalInput")
    w2_d = nc.dram_tensor("w2", [FFN, HIDDEN], bf16, kind="ExternalInput")
    s_d = nc.dram_tensor("s", [S_PAD], f32, kind="ExternalInput")
    y_d = nc.dram_tensor("y", [C, HIDDEN], f32, kind="ExternalOutput")

    # DRAM views tiled for 128-partition DMA
    xT_v = xT_d.ap().rearrange("(dc p) c -> p dc c", p=128)
    w2_v = w2_d.ap().rearrange("(ft p) d -> p ft d", p=128)
    s_v = s_d.ap().rearrange("(j p) -> p j", p=128)

    with tile.TileContext(nc) as tc:
        with (
            tc.tile_pool(name="res", bufs=1) as res,
            tc.tile_pool(name="w13", bufs=2) as w13,
            tc.tile_pool(name="hp", bufs=2) as hp,
            tc.tile_pool(name="sil", bufs=4) as silp,
            tc.tile_pool(name="yo", bufs=4) as yop,
            tc.tile_pool(name="ps", bufs=2, space=bass.MemorySpace.PSUM) as ps,
            tc.tile_pool(name="yps", bufs=4, space=bass.MemorySpace.PSUM) as yps,
        ):
            # resident tensors.  DMA order matters: the PE's first work is
            # chunk-0 phase A, which needs xT[:, :, :chunk] and w1/w3 of
            # fg 0 — so those transfers go first; the 8MB w2 (needed only
            # for phase B, ~85us in) is interleaved across chunk-0's
            # f-group loop so it never blocks the critical path.
            xT = res.tile([128, DC, C], bf16, tag="xT")
            w2 = res.tile([128, FT, HIDDEN], bf16, tag="w2")
            s_sb = res.tile([128, S_PAD // 128], f32, tag="s")

            # HAM pre-warm: the first ~13us are DMA-bound (ring priming +
            # the first weight tiles), during which the PE would idle and
            # its clock gate stays at 1.2GHz.  A burst of throwaway
            # matmuls on a zeroed scratch tile keeps the activity monitor
            # busy so the real matmuls start at the full 2.4GHz.
            warm_sb = silp.tile([128, 128], bf16, tag="warm_in", bufs=1)
            nc.gpsimd.memset(warm_sb[:], 0.0)
            warm_ps = ps.tile([128, 128], f32, tag="ph1", name="warm_ps")
            # sized to span entry (~7.4us) -> first-weights-landed (~12.7us):
            # ~32 cold-clock matmuls warm the HAM, the rest idle-fill
            N_WARM = 68
            for i in range(N_WARM):
                nc.tensor.matmul(warm_ps[:], warm_sb[:], warm_sb[:],
                                 start=(i == 0), stop=(i == N_WARM - 1))

            nc.sync.dma_start(xT[:, :, 0:chunks[0]], xT_v[:, :, 0:chunks[0]])

            t0 = 0
            for t, chunk in enumerate(chunks):
                hT = hp.tile([128, FT, chunk], bf16, tag="hT")
                # ---- phase A: hT[f, t] for this token chunk ----
                for fg in range(FG):
                    if t == 0 and fg == 0:
                        # first weight tiles split in dc-halves (separate
                        # tiles: Tile deps are tile-granular) so the
                        # first accumulation starts after 1.75MB, not
                        # 2.75MB, of startup DMA
                        w1a = w13.tile([128, 4, 512], bf16, tag="w1a", bufs=1)
                        w3a = w13.tile([128, 4, 512], bf16, tag="w3a", bufs=1)
                        w1b = w13.tile([128, 4, 512], bf16, tag="w1b", bufs=1)
                        w3b = w13.tile([128, 4, 512], bf16, tag="w3b", bufs=1)
                        nc.sync.dma_start(w1a[:], w1_d.ap()[0][:, 0:4, :])
                        nc.sync.dma_start(w3a[:], w3_d.ap()[0][:, 0:4, :])
                        nc.sync.dma_start(w1b[:], w1_d.ap()[0][:, 4:DC, :])
                        nc.sync.dma_start(w3b[:], w3_d.ap()[0][:, 4:DC, :])
                        w1_parts = [(w1a, 0), (w1b, 4)]
                        w3_parts = [(w3a, 0), (w3b, 4)]
                    else:
                        w1_sb = w13.tile([128, DC, 512], bf16, tag="w1")
                        w3_sb = w13.tile([128, DC, 512], bf16, tag="w3")
                        nc.sync.dma_start(w1_sb[:], w1_d.ap()[fg])
                        nc.sync.dma_start(w3_sb[:], w3_d.ap()[fg])
                        w1_parts = [(w1_sb, 0)]
                        w3_parts = [(w3_sb, 0)]
                    if t == 0 and fg >= 1:
                        # stream w2 (1MB slices) strictly behind the
                        # critical early w1/w3 loads; all slices land
                        # before phase B needs them (~85us in)
                        for q in ([fg - 1] if fg < FG - 1 else [FG - 2, FG - 1]):
                            nc.sync.dma_start(w2[:, q * 4:(q + 1) * 4, :],
                                              w2_v[:, q * 4:(q + 1) * 4, :])
                        if fg == 6:
                            nc.sync.dma_start(s_sb[:], s_v)
                        if fg in (3, 5):
                            # prefetch the next two chunks' xT slices
                            tn = {3: 1, 5: 2}[fg]
                            if tn < n_chunks:
                                o = sum(chunks[:tn])
                                nc.sync.dma_start(
                                    xT[:, :, o:o + chunks[tn]],
                                    xT_v[:, :, o:o + chunks[tn]],
                                )
                    if t >= 1 and fg == 1 and t + 2 < n_chunks:
                        # chunks 3+ prefetched one-ahead from chunk t>=1
                        tn = t + 2
                        o = sum(chunks[:tn])
                        nc.sync.dma_start(
                            xT[:, :, o:o + chunks[tn]],
                            xT_v[:, :, o:o + chunks[tn]],
                        )

                    def _wslice(parts, dc):
                        for tile_, base in parts:
                            if base <= dc < base + tile_.shape[1]:
                                return tile_[:, dc - base, :]
                        raise AssertionError(dc)

                    for fl in range(4):
                        ft = fg * 4 + fl
                        ph1 = ps.tile([128, chunk], f32, tag="ph1")
                        ph3 = ps.tile([128, chunk], f32, tag="ph3")
                        for dc in range(DC):
                            nc.tensor.matmul(
                                ph1[:],
                                _wslice(w1_parts, dc)[:, fl * 128:(fl + 1) * 128],
                                xT[:, dc, t0:t0 + chunk],
                                start=(dc == 0), stop=(dc == DC - 1),
                            )
                        for dc in range(DC):
                            nc.tensor.matmul(
                                ph3[:],
                                _wslice(w3_parts, dc)[:, fl * 128:(fl + 1) * 128],
                                xT[:, dc, t0:t0 + chunk],
                                start=(dc == 0), stop=(dc == DC - 1),
                            )
                        sil = silp.tile([128, chunk], bf16, tag="sil")
                        nc.scalar.activation(
                            sil[:], ph1[:], mybir.ActivationFunctionType.Silu
                        )
                        nc.vector.tensor_mul(hT[:, ft, :], sil[:], ph3[:])

                # ---- phase B: y[t, d] for this chunk ----
                # do-outer so do=0's scale+store overlaps do=1's matmuls
                # (shortens the kernel-exit tail after the last matmul)
                subs = [128] * (chunk // 128)
                if chunk % 128:
                    subs.append(chunk % 128)
                for tsub, tsz in enumerate(subs):
                    j = (t0 + tsub * 128) // 128
                    o = tsub * 128
                    for do in range(DO):
                        yp = yps.tile([128, 512], f32, tag="yp",
                                      name=f"yp{j}_{do}")
                        for f in range(FT):
                            nc.tensor.matmul(
                                yp[0:tsz, :],
                                hT[:, f, o:o + tsz],
                                w2[:, f, do * 512:(do + 1) * 512],
                                start=(f == 0), stop=(f == FT - 1),
                            )
                        ysb = yop.tile([128, 512], f32, tag="ysb")
                        # scale on ScalarE: out = in * s (per-partition)
                        nc.scalar.activation(
                            ysb[0:tsz, :], yp[0:tsz, :],
                            mybir.ActivationFunctionType.Copy,
                            scale=s_sb[0:tsz, j:j + 1],
                        )
                        nc.sync.dma_start(
                            y_d[t0 + o:t0 + o + tsz,
                                do * 512:(do + 1) * 512],
                            ysb[0:tsz, :],
                        )
                t0 += chunk
    nc.compile()
    return nc


def _get_nc(C, chunks):
    key = (C, chunks)
    if key not in _nc_cache:
        _nc_cache[key] = _build(C, chunks)
    return _nc_cache[key]


def _capacity(max_load):
    """Capacity C (multiple of 16) split into chunks: all but the last are
    multiples of 128 in [240, 512]; the last is a multiple of 16 kept
    >=240 when possible (below ~233 tokens phase-A matmuls go
    LDWEIGHTS-bound, wasting more than the padding saves)."""
    C = -(-max_load // 16) * 16
    n = max(1, -(-C // 512))
    chunks = []
    rem = C
    for i in range(n - 1):
        c = min(512, -(-rem // ((n - i) * 128)) * 128)
        chunks.append(c)
        rem -= c
    # keep the ragged tail >= 240 by shrinking earlier 128-mult chunks
    while n > 1 and rem < 240 and chunks:
        for i in range(len(chunks)):
            if rem >= 240:
                break
            if chunks[i] > 256:
                chunks[i] -= 128
                rem += 128
        else:
            break
    chunks.append(rem)
    assert sum(chunks) == C and all(c > 0 for c in chunks)
    return C, tuple(chunks)


def _pretile_w13(w):
    """[HIDDEN, FFN] -> [fg, p, dc, 512] bf16 (see _build's w1_d layout)."""
    w4 = np.ascontiguousarray(w).reshape(HIDDEN // 128, 128, FFN // 512, 512)
    return np.ascontiguousarray(w4.transpose(2, 1, 0, 3)).astype(_BF16)


# ---------------------------------------------------------------- kernel ----
def kernel(hidden_states, gate_w, gate_b, w1, w3, w2, _trace=False):
    from concourse.bass_utils import run_bass_kernel_spmd

    B, S, D = hidden_states.shape
    T = B * S
    x = np.asarray(hidden_states, np.float32).reshape(T, D)
    idxs, wts = _route(x, np.asarray(gate_w, np.float32),
                       np.asarray(gate_b, np.float32))
    C, chunks = _capacity(max(len(i) for i in idxs))
    S_PAD = -(-C // 128) * 128
    nc = _get_nc(C, chunks)

    w1 = np.asarray(w1)
    w3 = np.asarray(w3)
    w2 = np.asarray(w2)
    in_maps = []
    for e in range(NUM_EXPERTS):
        tok, wt = idxs[e], wts[e]
        l = len(tok)
        xT = np.zeros((D, C), _BF16)
        xT[:, :l] = x[tok].T.astype(_BF16)
        s = np.zeros((S_PAD,), np.float32)
        s[:l] = wt
        in_maps.append({
            "xT": xT,
            "w1": _pretile_w13(w1[e]),
            "w3": _pretile_w13(w3[e]),
            "w2": np.ascontiguousarray(w2[e]).astype(_BF16),
            "s": s,
        })

    res = run_bass_kernel_spmd(nc, in_maps, core_ids=list(range(N_CORES)),
                               trace=_trace)

    out = np.zeros((T, D), np.float32)
    for e in range(NUM_EXPERTS):
        tok = idxs[e]
        out[tok] += res.results[e]["y"][: len(tok)]
    out = out.reshape(B, S, D)
    if _trace:
        return out, res
    return out
